# revision 7
# baseline (speedup 1.0000x reference)
"""CompressedLinear (int8 weight, per-row scale) on 8 Trainium2 NeuronCores.

Math: y[b,s,o] = sum_i x[b,s,i] * (w_int8[o,i] * scale[o]) + bias[o]

Strategy (tensor-parallel over out_features, per sharding hint):
  - Shard W/scale/bias rows across 8 cores (1376 rows each); x replicated.
  - Scale is applied to the matmul OUTPUT (algebraically identical), so the
    device matmuls run on the raw int8 weights cast to fp16 (int8 is exact
    in fp16).
  - Single fp16 matmul pass: casting x to fp16 (on the HOST, which halves
    the x wire traffic) bounds the output relative error at ~2e-4.
  - Each core computes yT[o_shard, s] = W_shard @ x^T; the host hands each
    core pre-transposed views: xt = x^T [4096, 2048] fp16 and
    wt = W_shard^T [4096, 1376] int8.
  - DMA *instruction issue* costs ~0.7-1.0us per dma_start on a queue, and
    the PE consumes one (x k-slice, w k-slice) pair per ~0.9us during a
    PSUM-group sweep. So loads are batched into multi-k-slice blocks
    (3D access patterns) and the three w o-group streams go to three
    different engine queues (gpsimd/scalar/vector) so no single queue's
    issue rate is the bottleneck.
  - Per-partition affine (scale, bias) is fused into the PSUM eviction.
  - The very last PSUM group runs kt-inner per o-tile so the final
    evictions/output DMAs stagger into the matmul stream instead of
    serializing after the last matmul.
"""

import os
import numpy as np

import concourse.bass as bass
import concourse.tile as tile
from concourse import bacc, mybir
from concourse.bass_utils import run_bass_kernel_spmd

B = 1
S = 2048
I = 4096
O = 11008
N_CORES = 8
O_SHARD = O // N_CORES  # 1376
S_CHUNK = 512
P = 128
KB0 = 4   # k-slices per x block, chunk 0 (fine-grained for fast start)
KB = 8    # k-slices per x block, chunks 1+


def build_bass(I_=I, O_SHARD_=O_SHARD, S_=S, S_CHUNK_=S_CHUNK):
    KT = I_ // P
    N_CHUNKS = S_ // S_CHUNK_
    OT = (O_SHARD_ + P - 1) // P

    MM_DT = mybir.dt.float16
    nc = bacc.Bacc("TRN2", target_bir_lowering=False, debug=False)

    xt = nc.dram_tensor("xt", [I_, S_], mybir.dt.float16, kind="ExternalInput").ap()
    wt = nc.dram_tensor("wt", [I_, O_SHARD_], mybir.dt.int8, kind="ExternalInput").ap()
    scale = nc.dram_tensor("scale", [O_SHARD_], mybir.dt.float32, kind="ExternalInput").ap()
    bias = nc.dram_tensor("bias", [O_SHARD_], mybir.dt.float32, kind="ExternalInput").ap()
    yt = nc.dram_tensor("yt", [O_SHARD_, S_], mybir.dt.float32, kind="ExternalOutput").ap()

    # PSUM bank groups: 4+4+3 o-tiles so two adjacent groups fit in the
    # 8 banks and group transitions never wait on drains.
    groups = []
    g0 = 0
    for gsz in (4, 4, 3):
        if g0 < OT:
            groups.append((g0, min(g0 + gsz, OT)))
            g0 += gsz
    # w group g load stream goes to queue w_queues[g]; group 0 feeds the
    # first PSUM sweep so it gets the dedicated gpsimd queue.
    full_t = O_SHARD_ // P
    rem = O_SHARD_ - full_t * P

    with tile.TileContext(nc) as tc:
        with (
            tc.tile_pool(name="wres", bufs=1) as wres_pool,
            tc.tile_pool(name="consts", bufs=1) as const_pool,
            tc.tile_pool(name="xc0", bufs=KT // KB0) as x0_pool,
            tc.tile_pool(name="xcn", bufs=2 * (KT // KB)) as xn_pool,
            tc.tile_pool(name="outp", bufs=4) as out_pool,
            tc.tile_pool(name="psum", bufs=8, space="PSUM") as psum_pool,
        ):
            # PE warm-up: dependency-free matmuls on a zeroed tile keep the
            # PE busy from right after the preamble, so the HAM clock gate
            # opens (K=8/8) around when the first real matmuls flow.
            warm_sb = const_pool.tile([P, P], MM_DT)
            nc.any.memset(warm_sb[:], 0.0)
            warm_ps = psum_pool.tile([P, P], mybir.dt.float32, name="warm_ps", tag="psum")
            N_WARM = 12
            for i in range(N_WARM):
                nc.tensor.matmul(
                    warm_ps[:], warm_sb[:], warm_sb[:],
                    start=(i == 0), stop=(i == N_WARM - 1),
                )

            # chunk-0 x blocks: KB0 k-slices per DMA on the sync queue,
            # nothing queued ahead of them.
            def emit_x_chunk(sc):
                s0 = sc * S_CHUNK_
                kb = KB0 if sc == 0 else KB
                pool = x0_pool if sc == 0 else xn_pool
                blocks = []
                for b in range(KT // kb):
                    bt = pool.tile([P, kb * S_CHUNK_], MM_DT, tag=f"xb{kb}")
                    src = xt[b * kb * P:(b + 1) * kb * P, s0:s0 + S_CHUNK_]
                    nc.sync.dma_start(
                        bt[:].rearrange("p (kt s) -> p kt s", s=S_CHUNK_),
                        src.rearrange("(kt p) s -> p kt s", p=P))
                    blocks.append(bt)
                # rhs view for k-slice kt
                def rhs(kt, blocks=blocks, kb=kb):
                    return blocks[kt // kb][:, (kt % kb) * S_CHUNK_:(kt % kb + 1) * S_CHUNK_]
                return rhs

            # per-partition scale/bias columns on the scalar queue (tiny;
            # must not delay the x stream on sync).
            scale_t = const_pool.tile([P, OT], mybir.dt.float32)
            bias_t = const_pool.tile([P, OT], mybir.dt.float32)
            if full_t:
                nc.scalar.dma_start(
                    scale_t[:, :full_t], scale[: full_t * P].rearrange("(t p) -> p t", p=P)
                )
                nc.scalar.dma_start(
                    bias_t[:, :full_t], bias[: full_t * P].rearrange("(t p) -> p t", p=P)
                )
            if rem:
                nc.scalar.dma_start(
                    scale_t[:rem, full_t:], scale[full_t * P:].rearrange("(t p) -> p t", p=rem)
                )
                nc.scalar.dma_start(
                    bias_t[:rem, full_t:], bias[full_t * P:].rearrange("(t p) -> p t", p=rem)
                )

            rhs0 = emit_x_chunk(0)

            # Weight shard int8 -> fp16 (exact), kept resident in SBUF.
            # Casting DMAs can only run on the gpsimd (SWDGE) queue, and
            # each dma_start costs ~1us of queue issue time, so batch
            # KWB k-slices per DMA (3D access pattern) and stream the
            # three o-groups in PSUM-sweep order, kt-ordered within each.
            KWB = 4
            w_blocks = [None] * len(groups)
            w_widths = [None] * len(groups)
            for g, (g_start, g_end) in enumerate(groups):
                c0 = g_start * P
                c1 = min(g_end * P, O_SHARD_)
                wid = c1 - c0
                w_widths[g] = wid
                blocks = []
                for b in range(KT // KWB):
                    w_b = wres_pool.tile([P, KWB * wid], MM_DT, tag=f"wb{g}_{b}")
                    src = wt[b * KWB * P:(b + 1) * KWB * P, c0:c1]
                    nc.gpsimd.dma_start(
                        w_b[:].rearrange("p (kt o) -> p kt o", o=wid),
                        src.rearrange("(kt p) o -> p kt o", p=P))
                    blocks.append(w_b)
                w_blocks[g] = blocks

            def w_slice_for(kt, g, ot_local, orows):
                wid = w_widths[g]
                base = (kt % KWB) * wid + ot_local * P
                return w_blocks[g][kt // KWB][:, base:base + orows]

            def evict(sc, ot, psum_t):
                s0 = sc * S_CHUNK_
                orows = min(P, O_SHARD_ - ot * P)
                out_t = out_pool.tile([P, S_CHUNK_], mybir.dt.float32)
                nc.vector.tensor_scalar(
                    out=out_t[:orows, :],
                    in0=psum_t[:orows, :],
                    scalar1=scale_t[:orows, ot:ot + 1],
                    scalar2=bias_t[:orows, ot:ot + 1],
                    op0=mybir.AluOpType.mult,
                    op1=mybir.AluOpType.add,
                )
                nc.sync.dma_start(
                    yt[ot * P:ot * P + orows, s0:s0 + S_CHUNK_],
                    out_t[:orows, :],
                )

            def emit_groups(sc, rhs, tail=False):
                # kt outer / o-tile inner: each x block's last reader comes
                # early in the group sweep, so next-chunk loads spread over
                # the whole chunk instead of bunching at its tail.
                for g, (g_start, g_end) in enumerate(groups):
                    last_group = tail and g == len(groups) - 1
                    if last_group:
                        # kt-inner per o-tile: each o-tile completes ~7us
                        # apart, so evictions/output DMAs overlap the
                        # remaining matmuls instead of serializing at the
                        # very end of the kernel.
                        for ot in range(g_start, g_end):
                            orows = min(P, O_SHARD_ - ot * P)
                            ps = psum_pool.tile(
                                [P, S_CHUNK_], mybir.dt.float32,
                                name=f"psum_{sc}_{ot}", tag="psum",
                            )
                            for kt in range(KT):
                                w_slice = w_slice_for(kt, g, ot - g_start, orows)
                                nc.tensor.matmul(
                                    ps[:orows, :], w_slice, rhs(kt),
                                    start=(kt == 0), stop=(kt == KT - 1),
                                )
                            evict(sc, ot, ps)
                        continue
                    psums = {}
                    for ot in range(g_start, g_end):
                        psums[ot] = psum_pool.tile(
                            [P, S_CHUNK_], mybir.dt.float32,
                            name=f"psum_{sc}_{ot}", tag="psum",
                        )
                    for kt in range(KT):
                        for ot in range(g_start, g_end):
                            orows = min(P, O_SHARD_ - ot * P)
                            w_slice = w_slice_for(kt, g, ot - g_start, orows)
                            nc.tensor.matmul(
                                psums[ot][:orows, :], w_slice, rhs(kt),
                                start=(kt == 0), stop=(kt == KT - 1),
                            )
                    for ot in range(g_start, g_end):
                        evict(sc, ot, psums[ot])

            # Software-pipelined emission: loads for chunk sc+1 are emitted
            # before chunk sc's matmul groups, so in the per-queue FIFO
            # streams next-chunk loads sit ahead of this chunk's PSUM
            # drains.
            prev = rhs0
            for sc in range(N_CHUNKS):
                if sc + 1 < N_CHUNKS:
                    nxt = emit_x_chunk(sc + 1)
                else:
                    nxt = None
                emit_groups(sc, prev, tail=(sc == N_CHUNKS - 1))
                prev = nxt

    nc.compile()
    return nc


_NC_CACHE = None


def _get_nc():
    global _NC_CACHE
    if _NC_CACHE is None:
        _NC_CACHE = build_bass()
    return _NC_CACHE


def run(inputs, trace=False, trace_cores=None, tmpdir=None):
    x = np.asarray(inputs["x"])
    w = np.asarray(inputs["weight_int8"])
    scale = np.asarray(inputs["scale"], dtype=np.float32)
    bias = np.asarray(inputs["bias"], dtype=np.float32)

    if w.dtype != np.int8:
        w = w.astype(np.int8)
    x2d = np.ascontiguousarray(x.reshape(S, I).astype(np.float32, copy=False))
    xtr = np.ascontiguousarray(x2d.T.astype(np.float16))  # [I, S] fp16

    in_maps = []
    for c in range(N_CORES):
        sl = slice(c * O_SHARD, (c + 1) * O_SHARD)
        in_maps.append({
            "xt": xtr,
            "wt": np.ascontiguousarray(w[sl, :].T),  # [I, O_SHARD]
            "scale": np.ascontiguousarray(scale[sl]),
            "bias": np.ascontiguousarray(bias[sl]),
        })

    nc = _get_nc()
    kwargs = {}
    if trace:
        kwargs["trace"] = True
        if trace_cores is not None:
            kwargs["trace_cores"] = trace_cores
        if tmpdir is not None:
            kwargs["tmpdir"] = tmpdir
    res = run_bass_kernel_spmd(nc, in_maps, core_ids=list(range(N_CORES)), **kwargs)

    yt_full = np.concatenate([res.results[c]["yt"] for c in range(N_CORES)], axis=0)
    out = np.ascontiguousarray(yt_full.T).reshape(B, S, O).astype(np.float32, copy=False)
    if trace:
        return out, res
    return out


def kernel(**inputs) -> np.ndarray:
    return run(inputs, trace=False)
